# revision 7
# baseline (speedup 1.0000x reference)
"""Trainium2 Bass kernel for bidirectional ActionLSTM.

Full inputs in, full output out. Internally: data-parallel over batch
(8 NeuronCores x 256 batch rows), LSTM weights replicated.

Device program (per core, transposed layout: hidden on partitions,
batch on the free dim):
  - fc_in is folded into the LSTM input weights on the host:
        W_x = w_ih @ fc_in_w  [512, 68],  bias = w_ih@fc_in_b + b_ih + b_hh
    and the bias is folded in as an extra all-ones input row (K=69).
  - Gate order permuted to [i, f, o, g]. tanh is computed via the
    identity tanh(z) = 2*sigmoid(2z) - 1, with the 2z folded into the
    g-gate weight rows, so ONE sigmoid activation op covers all gates.
  - h is stored as h/2 ("h_half"); the 2x is folded into the recurrent
    and pooling weights. tanh(c) likewise becomes sigmoid(2c) via the
    activation's free scale.
  - Mean-pool over time + fc_out are folded into a per-step accumulating
    matmul into PSUM: pacc += (2/T * Wo_dir) @ h_half_t.
Per (step, dir): 9 matmuls (4 x-side K=69 + 4 recurrent K=128 + 1 pool),
1 big sigmoid [128,1024], 1 sigmoid(2c) [128,256] on ACT, 3
scalar_tensor_tensor ops on DVE, 1 tensor_tensor on GpSimd.
"""

import os
import numpy as np
from contextlib import ExitStack

INPUT, HID, NCLS = 68, 128, 3
B, T = 2048, 128
NCORES = 8
BL = B // NCORES          # 256 batch rows per core
KX = INPUT + 1            # 69 (ones row folds bias in)
G4 = 4 * HID              # 512

# matmul operand dtype: "f32r" (full fp32 storage, fast tensor-engine mode)
# or "bf16"
MM_DTYPE = os.environ.get("LSTM_MM_DTYPE", "f32r")

_CACHE = {}


def _build_program():
    import concourse.bass as bass
    import concourse.tile as tile
    from concourse import bacc, mybir

    f32 = mybir.dt.float32
    AF = mybir.ActivationFunctionType
    OP = mybir.AluOpType
    use_bf16 = MM_DTYPE == "bf16"
    # dtype for tensors consumed by the tensor engine (x, weights, h):
    # float32r = fp32 storage, full-speed matmul (walrus requires the
    # producing instruction's output dtype to be float32r itself).
    sb_dt = mybir.dt.bfloat16 if use_bf16 else mybir.dt.float32r

    def R(ap):
        return ap

    nc = bacc.Bacc("TRN2", target_bir_lowering=False, debug=False,
                   num_devices=NCORES)

    xin = nc.dram_tensor("xin", [KX, T * BL], sb_dt, kind="ExternalInput").ap()
    wx = {}
    wu = {}
    wo = {}
    for d in "fb":
        wx[d] = nc.dram_tensor(f"wx_{d}", [KX, G4], sb_dt,
                               kind="ExternalInput").ap()
        wu[d] = nc.dram_tensor(f"wu_{d}", [HID, G4], sb_dt,
                               kind="ExternalInput").ap()
        wo[d] = nc.dram_tensor(f"wo_{d}", [HID, NCLS], sb_dt,
                               kind="ExternalInput").ap()
    out = nc.dram_tensor("out", [NCLS, BL], f32, kind="ExternalOutput").ap()

    with tile.TileContext(nc) as tc, ExitStack() as ctx:
        const = ctx.enter_context(tc.tile_pool(name="const", bufs=1))
        X = const.tile([KX, T * BL], sb_dt, tag="X")
        # split the big input DMA into chunks so it spreads across DMA
        # queues and so early timesteps unblock compute quickly; issue
        # from both ends since the bwd direction consumes t=T-1 first.
        NCHUNK = 16
        CW = T * BL // NCHUNK
        order = []
        for i in range(NCHUNK // 2):
            order += [NCHUNK - 1 - i, i]
        for ci in order:
            nc.sync.dma_start(X[:, ci * CW:(ci + 1) * CW],
                              xin[:, ci * CW:(ci + 1) * CW])

        WX = {}
        WU = {}
        WO = {}
        for d in "fb":
            WX[d] = const.tile([KX, G4], sb_dt, tag=f"wx{d}", name=f"WX{d}")
            nc.sync.dma_start(WX[d][:], wx[d][:])
            WU[d] = const.tile([HID, G4], sb_dt, tag=f"wu{d}", name=f"WU{d}")
            nc.sync.dma_start(WU[d][:], wu[d][:])
            WO[d] = const.tile([HID, NCLS], sb_dt, tag=f"wo{d}", name=f"WO{d}")
            nc.sync.dma_start(WO[d][:], wo[d][:])

        hpool = ctx.enter_context(tc.tile_pool(name="h", bufs=3))
        cpool = ctx.enter_context(tc.tile_pool(name="c", bufs=3))
        spool = ctx.enter_context(tc.tile_pool(name="s", bufs=2))
        scpool = ctx.enter_context(tc.tile_pool(name="sc", bufs=2))
        mpool = ctx.enter_context(tc.tile_pool(name="m1h", bufs=2))
        tpool = ctx.enter_context(tc.tile_pool(name="tt", bufs=2))
        gpsum = ctx.enter_context(tc.tile_pool(name="gates", bufs=1,
                                               space="PSUM"))
        ppsum = ctx.enter_context(tc.tile_pool(name="pacc", bufs=1,
                                               space="PSUM"))

        h = {}
        c = {}
        pacc = {}
        for d in "fb":
            h[d] = hpool.tile([HID, BL], sb_dt, tag=f"h{d}", name=f"h0{d}")
            nc.vector.memset(h[d][:].bitcast(f32) if not use_bf16 else h[d][:],
                             0.0)
            c[d] = cpool.tile([HID, BL], f32, tag=f"c{d}", name=f"c0{d}")
            nc.vector.memset(c[d][:], 0.0)
            pacc[d] = ppsum.tile([NCLS, BL], f32, tag=f"p{d}", name=f"pacc{d}")

        for t in range(T):
            for d in ("f", "b"):
                xi = t if d == "f" else T - 1 - t
                xs = X[:, xi * BL:(xi + 1) * BL]
                g = gpsum.tile([HID, 4 * BL], f32, tag=f"g{d}", name=f"g_{d}_{t}")
                for gi in range(4):
                    gslice = g[:, gi * BL:(gi + 1) * BL]
                    nc.tensor.matmul(gslice,
                                     R(WX[d][:, gi * HID:(gi + 1) * HID]),
                                     R(xs), start=True, stop=False)
                    nc.tensor.matmul(gslice,
                                     R(WU[d][:, gi * HID:(gi + 1) * HID]),
                                     R(h[d][:]), start=False, stop=True)
                # s = sigmoid over all four gates (g-gate pre-doubled)
                s = spool.tile([HID, 4 * BL], f32, tag=f"s{d}")
                nc.scalar.activation(s[:], g[:], AF.Sigmoid)
                s_i = s[:, 0:BL]
                s_f = s[:, BL:2 * BL]
                s_o = s[:, 2 * BL:3 * BL]
                s_g = s[:, 3 * BL:4 * BL]
                # tt = sigmoid(f) * c_prev   (on GpSimd, off DVE's back)
                tt = tpool.tile([HID, BL], f32, tag=f"tt{d}")
                nc.gpsimd.tensor_tensor(tt[:], s_f, c[d][:], op=OP.mult)
                # m1h = (s_g - 0.5) * s_i  == 0.5*sigmoid(i)*tanh(g)
                m1h = mpool.tile([HID, BL], f32, tag=f"m{d}")
                nc.vector.scalar_tensor_tensor(m1h[:], s_g, 0.5, s_i,
                                               op0=OP.subtract, op1=OP.mult)
                # c = 2*m1h + tt
                c_new = cpool.tile([HID, BL], f32, tag=f"c{d}")
                nc.vector.scalar_tensor_tensor(c_new[:], m1h[:], 2.0, tt[:],
                                               op0=OP.mult, op1=OP.add)
                # sc = sigmoid(2c);  h_half = (sc - 0.5) * s_o == h/2
                sc = scpool.tile([HID, BL], f32, tag=f"sc{d}")
                nc.scalar.activation(sc[:], c_new[:], AF.Sigmoid, scale=2.0)
                h_new = hpool.tile([HID, BL], sb_dt, tag=f"h{d}")
                nc.vector.scalar_tensor_tensor(h_new[:], sc[:], 0.5, s_o,
                                               op0=OP.subtract, op1=OP.mult)
                # pool: pacc += (2/T*Wo) @ h_half
                nc.tensor.matmul(pacc[d][:], R(WO[d][:]), R(h_new[:]),
                                 start=(t == 0), stop=(t == T - 1),
                                 skip_group_check=True)
                h[d] = h_new
                c[d] = c_new

        osum = const.tile([NCLS, BL], f32, tag="osum")
        nc.scalar.copy(osum[:], pacc["f"][:])
        nc.vector.tensor_tensor(osum[:], osum[:], pacc["b"][:], op=OP.add)
        nc.sync.dma_start(out[:], osum[:])

    nc.compile()
    return nc


def _prep_weights(w_ih, w_hh, b_ih, b_hh, fc_in_w, fc_in_b):
    Wx = w_ih.astype(np.float64) @ fc_in_w.astype(np.float64)   # [512, 68]
    bias = w_ih.astype(np.float64) @ fc_in_b.astype(np.float64) \
        + b_ih.astype(np.float64) + b_hh.astype(np.float64)
    perm = np.concatenate([np.arange(0, 128), np.arange(128, 256),
                           np.arange(384, 512), np.arange(256, 384)])
    Wx = Wx[perm]
    U = w_hh.astype(np.float64)[perm]
    bias = bias[perm]
    srow = np.ones((512, 1), np.float64)
    srow[384:] = 2.0
    Wx_aug = np.concatenate([Wx, bias[:, None]], axis=1)        # [512, 69]
    lhsT_x = np.ascontiguousarray((srow * Wx_aug).T)            # [69, 512]
    lhsT_U = np.ascontiguousarray((srow * U * 2.0).T)           # [128, 512]
    return lhsT_x, lhsT_U


def kernel(x, fc_in_w, fc_in_b, w_ih_f, w_hh_f, b_ih_f, b_hh_f,
           w_ih_b, w_hh_b, b_ih_b, b_hh_b, fc_out_w, fc_out_b,
           _want_trace=False):
    from concourse import bass_utils

    np_dt = np.float32
    if MM_DTYPE == "bf16":
        import ml_dtypes
        np_dt = ml_dtypes.bfloat16

    if "nc" not in _CACHE:
        _CACHE["nc"] = _build_program()
    nc = _CACHE["nc"]

    lx_f, lU_f = _prep_weights(w_ih_f, w_hh_f, b_ih_f, b_hh_f,
                               fc_in_w, fc_in_b)
    lx_b, lU_b = _prep_weights(w_ih_b, w_hh_b, b_ih_b, b_hh_b,
                               fc_in_w, fc_in_b)
    wo_f = np.ascontiguousarray((2.0 / T) * fc_out_w[:, :HID].astype(np.float64).T)
    wo_b = np.ascontiguousarray((2.0 / T) * fc_out_w[:, HID:].astype(np.float64).T)

    shared = {
        "wx_f": lx_f.astype(np_dt), "wu_f": lU_f.astype(np_dt),
        "wx_b": lx_b.astype(np_dt), "wu_b": lU_b.astype(np_dt),
        "wo_f": wo_f.astype(np_dt), "wo_b": wo_b.astype(np_dt),
    }
    in_maps = []
    for cidx in range(NCORES):
        xs = x[cidx * BL:(cidx + 1) * BL]                    # [BL, T, 68]
        xT = np.ascontiguousarray(xs.transpose(2, 1, 0))     # [68, T, BL]
        x_aug = np.concatenate(
            [xT, np.ones((1, T, BL), np.float32)], axis=0)   # [69, T, BL]
        in_maps.append({"xin": x_aug.reshape(KX, T * BL).astype(np_dt),
                        **shared})

    res = bass_utils.run_bass_kernel_spmd(
        nc, in_maps, core_ids=list(range(NCORES)), trace=_want_trace)
    outs = []
    for cidx in range(NCORES):
        o = res.results[cidx]["out"]                          # [3, BL]
        out_core = o.T + fc_out_b                             # [BL, 3]
        outs.append(out_core)
    full = np.concatenate(outs, axis=0).astype(np.float32)
    if _want_trace:
        _CACHE["last_result"] = res
    return full


# revision 8
# speedup vs baseline: 1.0323x; 1.0323x over previous
"""Trainium2 Bass kernel for bidirectional ActionLSTM.

Full inputs in, full output out. Internally: data-parallel over batch
(8 NeuronCores x 256 batch rows), LSTM weights replicated.

Device program (per core, transposed layout: hidden on partitions,
batch on the free dim):
  - fc_in is folded into the LSTM input weights on the host:
        W_x = w_ih @ fc_in_w  [512, 68],  bias = w_ih@fc_in_b + b_ih + b_hh
    and the bias is folded in as an extra all-ones input row (K=69).
  - Gate order permuted to [i, f, o, g]. tanh is computed via the
    identity tanh(z) = 2*sigmoid(2z) - 1, with the 2z folded into the
    g-gate weight rows, so ONE sigmoid activation op covers all gates.
  - h is stored as h/2 ("h_half"); the 2x is folded into the recurrent
    and pooling weights. tanh(c) likewise becomes sigmoid(2c) via the
    activation's free scale.
  - Mean-pool over time + fc_out are folded into a per-step accumulating
    matmul into PSUM: pacc += (2/T * Wo_dir) @ h_half_t.
Per (step, dir): 9 matmuls (4 x-side K=69 + 4 recurrent K=128 + 1 pool),
1 big sigmoid [128,1024], 1 sigmoid(2c) [128,256] on ACT, 3
scalar_tensor_tensor ops on DVE, 1 tensor_tensor on GpSimd.
"""

import os
import numpy as np
from contextlib import ExitStack

INPUT, HID, NCLS = 68, 128, 3
B, T = 2048, 128
NCORES = 8
BL = B // NCORES          # 256 batch rows per core
KX = INPUT + 1            # 69 (ones row folds bias in)
G4 = 4 * HID              # 512

# matmul operand dtype: "f32r" (full fp32 storage, fast tensor-engine mode)
# or "bf16"
MM_DTYPE = os.environ.get("LSTM_MM_DTYPE", "bf16")

_CACHE = {}


def _build_program():
    import concourse.bass as bass
    import concourse.tile as tile
    from concourse import bacc, mybir

    f32 = mybir.dt.float32
    AF = mybir.ActivationFunctionType
    OP = mybir.AluOpType
    use_bf16 = MM_DTYPE == "bf16"
    # dtype for tensors consumed by the tensor engine (x, weights, h):
    # bf16 runs the matmul at full rate (fp32r measured at 1/4 rate on HW);
    # gate accumulation stays fp32 in PSUM, s/c stay fp32 on DVE/ACT.
    sb_dt = mybir.dt.bfloat16 if use_bf16 else mybir.dt.float32r

    def R(ap):
        return ap

    nc = bacc.Bacc("TRN2", target_bir_lowering=False, debug=False,
                   num_devices=NCORES)

    xin = nc.dram_tensor("xin", [KX, T * BL], sb_dt, kind="ExternalInput").ap()
    wx = {}
    wu = {}
    wo = {}
    for d in "fb":
        wx[d] = nc.dram_tensor(f"wx_{d}", [KX, G4], sb_dt,
                               kind="ExternalInput").ap()
        wu[d] = nc.dram_tensor(f"wu_{d}", [HID, G4], sb_dt,
                               kind="ExternalInput").ap()
        wo[d] = nc.dram_tensor(f"wo_{d}", [HID, NCLS], sb_dt,
                               kind="ExternalInput").ap()
    out = nc.dram_tensor("out", [NCLS, BL], f32, kind="ExternalOutput").ap()

    with tile.TileContext(nc) as tc, ExitStack() as ctx:
        const = ctx.enter_context(tc.tile_pool(name="const", bufs=1))
        X = const.tile([KX, T * BL], sb_dt, tag="X")
        # split the big input DMA into chunks so it spreads across DMA
        # queues and so early timesteps unblock compute quickly; issue
        # from both ends since the bwd direction consumes t=T-1 first.
        NCHUNK = 16
        CW = T * BL // NCHUNK
        order = []
        for i in range(NCHUNK // 2):
            order += [NCHUNK - 1 - i, i]
        for ci in order:
            nc.sync.dma_start(X[:, ci * CW:(ci + 1) * CW],
                              xin[:, ci * CW:(ci + 1) * CW])

        WX = {}
        WU = {}
        WO = {}
        for d in "fb":
            WX[d] = const.tile([KX, G4], sb_dt, tag=f"wx{d}", name=f"WX{d}")
            nc.sync.dma_start(WX[d][:], wx[d][:])
            WU[d] = const.tile([HID, G4], sb_dt, tag=f"wu{d}", name=f"WU{d}")
            nc.sync.dma_start(WU[d][:], wu[d][:])
            WO[d] = const.tile([HID, NCLS], sb_dt, tag=f"wo{d}", name=f"WO{d}")
            nc.sync.dma_start(WO[d][:], wo[d][:])

        hpool = ctx.enter_context(tc.tile_pool(name="h", bufs=3))
        cpool = ctx.enter_context(tc.tile_pool(name="c", bufs=3))
        spool = ctx.enter_context(tc.tile_pool(name="s", bufs=2))
        scpool = ctx.enter_context(tc.tile_pool(name="sc", bufs=2))
        mpool = ctx.enter_context(tc.tile_pool(name="m1h", bufs=2))
        tpool = ctx.enter_context(tc.tile_pool(name="tt", bufs=2))
        gpsum = ctx.enter_context(tc.tile_pool(name="gates", bufs=1,
                                               space="PSUM"))
        ppsum = ctx.enter_context(tc.tile_pool(name="pacc", bufs=1,
                                               space="PSUM"))

        h = {}
        c = {}
        pacc = {}
        for d in "fb":
            h[d] = hpool.tile([HID, BL], sb_dt, tag=f"h{d}", name=f"h0{d}")
            nc.vector.memset(h[d][:].bitcast(f32) if not use_bf16 else h[d][:],
                             0.0)
            c[d] = cpool.tile([HID, BL], f32, tag=f"c{d}", name=f"c0{d}")
            nc.vector.memset(c[d][:], 0.0)
            pacc[d] = ppsum.tile([NCLS, BL], f32, tag=f"p{d}", name=f"pacc{d}")

        for t in range(T):
            for d in ("f", "b"):
                xi = t if d == "f" else T - 1 - t
                xs = X[:, xi * BL:(xi + 1) * BL]
                g = gpsum.tile([HID, 4 * BL], f32, tag=f"g{d}", name=f"g_{d}_{t}")
                for gi in range(4):
                    gslice = g[:, gi * BL:(gi + 1) * BL]
                    nc.tensor.matmul(gslice,
                                     R(WX[d][:, gi * HID:(gi + 1) * HID]),
                                     R(xs), start=True, stop=False)
                    nc.tensor.matmul(gslice,
                                     R(WU[d][:, gi * HID:(gi + 1) * HID]),
                                     R(h[d][:]), start=False, stop=True)
                # s = sigmoid over all four gates (g-gate pre-doubled)
                s = spool.tile([HID, 4 * BL], f32, tag=f"s{d}")
                nc.scalar.activation(s[:], g[:], AF.Sigmoid)
                s_i = s[:, 0:BL]
                s_f = s[:, BL:2 * BL]
                s_o = s[:, 2 * BL:3 * BL]
                s_g = s[:, 3 * BL:4 * BL]
                # tt = sigmoid(f) * chalf_prev  (c stored as c/2)
                tt = tpool.tile([HID, BL], f32, tag=f"tt{d}")
                nc.vector.tensor_tensor(tt[:], s_f, c[d][:], op=OP.mult)
                # m1h = (s_g - 0.5) * s_i  == 0.5*sigmoid(i)*tanh(g)
                m1h = mpool.tile([HID, BL], f32, tag=f"m{d}")
                nc.vector.scalar_tensor_tensor(m1h[:], s_g, 0.5, s_i,
                                               op0=OP.subtract, op1=OP.mult)
                # chalf = m1h + tt   (== c/2: m1h is 0.5*i*tanh(g),
                # tt is sig(f)*c_prev/2)
                c_new = cpool.tile([HID, BL], f32, tag=f"c{d}")
                nc.vector.tensor_tensor(c_new[:], m1h[:], tt[:], op=OP.add)
                # sc = sigmoid(4*chalf) = sigmoid(2c)
                sc = scpool.tile([HID, BL], f32, tag=f"sc{d}")
                nc.scalar.activation(sc[:], c_new[:], AF.Sigmoid, scale=4.0)
                h_new = hpool.tile([HID, BL], sb_dt, tag=f"h{d}")
                nc.vector.scalar_tensor_tensor(h_new[:], sc[:], 0.5, s_o,
                                               op0=OP.subtract, op1=OP.mult)
                # pool: pacc += (2/T*Wo) @ h_half
                nc.tensor.matmul(pacc[d][:], R(WO[d][:]), R(h_new[:]),
                                 start=(t == 0), stop=(t == T - 1),
                                 skip_group_check=True)
                h[d] = h_new
                c[d] = c_new

        osum = const.tile([NCLS, BL], f32, tag="osum")
        nc.scalar.copy(osum[:], pacc["f"][:])
        nc.vector.tensor_tensor(osum[:], osum[:], pacc["b"][:], op=OP.add)
        nc.sync.dma_start(out[:], osum[:])

    nc.compile()
    return nc


def _prep_weights(w_ih, w_hh, b_ih, b_hh, fc_in_w, fc_in_b):
    Wx = w_ih.astype(np.float64) @ fc_in_w.astype(np.float64)   # [512, 68]
    bias = w_ih.astype(np.float64) @ fc_in_b.astype(np.float64) \
        + b_ih.astype(np.float64) + b_hh.astype(np.float64)
    perm = np.concatenate([np.arange(0, 128), np.arange(128, 256),
                           np.arange(384, 512), np.arange(256, 384)])
    Wx = Wx[perm]
    U = w_hh.astype(np.float64)[perm]
    bias = bias[perm]
    srow = np.ones((512, 1), np.float64)
    srow[384:] = 2.0
    Wx_aug = np.concatenate([Wx, bias[:, None]], axis=1)        # [512, 69]
    lhsT_x = np.ascontiguousarray((srow * Wx_aug).T)            # [69, 512]
    lhsT_U = np.ascontiguousarray((srow * U * 2.0).T)           # [128, 512]
    return lhsT_x, lhsT_U


def kernel(x, fc_in_w, fc_in_b, w_ih_f, w_hh_f, b_ih_f, b_hh_f,
           w_ih_b, w_hh_b, b_ih_b, b_hh_b, fc_out_w, fc_out_b,
           _want_trace=False):
    from concourse import bass_utils

    np_dt = np.float32
    if MM_DTYPE == "bf16":
        import ml_dtypes
        np_dt = ml_dtypes.bfloat16

    if "nc" not in _CACHE:
        _CACHE["nc"] = _build_program()
    nc = _CACHE["nc"]

    lx_f, lU_f = _prep_weights(w_ih_f, w_hh_f, b_ih_f, b_hh_f,
                               fc_in_w, fc_in_b)
    lx_b, lU_b = _prep_weights(w_ih_b, w_hh_b, b_ih_b, b_hh_b,
                               fc_in_w, fc_in_b)
    wo_f = np.ascontiguousarray((2.0 / T) * fc_out_w[:, :HID].astype(np.float64).T)
    wo_b = np.ascontiguousarray((2.0 / T) * fc_out_w[:, HID:].astype(np.float64).T)

    shared = {
        "wx_f": lx_f.astype(np_dt), "wu_f": lU_f.astype(np_dt),
        "wx_b": lx_b.astype(np_dt), "wu_b": lU_b.astype(np_dt),
        "wo_f": wo_f.astype(np_dt), "wo_b": wo_b.astype(np_dt),
    }
    in_maps = []
    for cidx in range(NCORES):
        xs = x[cidx * BL:(cidx + 1) * BL]                    # [BL, T, 68]
        xT = np.ascontiguousarray(xs.transpose(2, 1, 0))     # [68, T, BL]
        x_aug = np.concatenate(
            [xT, np.ones((1, T, BL), np.float32)], axis=0)   # [69, T, BL]
        in_maps.append({"xin": x_aug.reshape(KX, T * BL).astype(np_dt),
                        **shared})

    res = bass_utils.run_bass_kernel_spmd(
        nc, in_maps, core_ids=list(range(NCORES)), trace=_want_trace)
    outs = []
    for cidx in range(NCORES):
        o = res.results[cidx]["out"]                          # [3, BL]
        out_core = o.T + fc_out_b                             # [BL, 3]
        outs.append(out_core)
    full = np.concatenate(outs, axis=0).astype(np.float32)
    if _want_trace:
        _CACHE["last_result"] = res
    return full


# revision 11
# speedup vs baseline: 1.1643x; 1.1279x over previous
"""Trainium2 Bass kernel for bidirectional ActionLSTM.

Full inputs in, full output out. Internally: data-parallel over batch
(8 NeuronCores x 256 batch rows), LSTM weights replicated.

Device program (per core, transposed layout: hidden on partitions,
batch on the free dim):
  - fc_in is folded into the LSTM input weights on the host:
        W_x = w_ih @ fc_in_w  [512, 68],  bias = w_ih@fc_in_b + b_ih + b_hh
    and the bias is folded in as an extra all-ones input row (K=69).
  - Gate order permuted to [i, f, o, g]. tanh is computed via the
    identity tanh(z) = 2*sigmoid(2z) - 1, with the 2z folded into the
    g-gate weight rows, so ONE sigmoid activation op covers all gates.
  - h is stored as h/2 ("h_half"); the 2x is folded into the recurrent
    and pooling weights. tanh(c) likewise becomes sigmoid(2c) via the
    activation's free scale.
  - Mean-pool over time + fc_out are folded into a per-step accumulating
    matmul into PSUM: pacc += (2/T * Wo_dir) @ h_half_t.
Per (step, dir): 9 matmuls (4 x-side K=69 + 4 recurrent K=128 + 1 pool),
1 big sigmoid [128,1024], 1 sigmoid(2c) [128,256] on ACT, 3
scalar_tensor_tensor ops on DVE, 1 tensor_tensor on GpSimd.
"""

import os
import numpy as np
from contextlib import ExitStack

INPUT, HID, NCLS = 68, 128, 3
B, T = 2048, 128
NCORES = 8
BL = B // NCORES          # 256 batch rows per core
KX = INPUT + 1            # 69 (ones row folds bias in)
G4 = 4 * HID              # 512

# matmul operand dtype: "f32r" (full fp32 storage, fast tensor-engine mode)
# or "bf16"
MM_DTYPE = os.environ.get("LSTM_MM_DTYPE", "bf16")
USE_FILLER = os.environ.get("LSTM_FILLER", "1") == "1"

_CACHE = {}


def _build_program():
    import concourse.bass as bass
    import concourse.tile as tile
    from concourse import bacc, mybir

    f32 = mybir.dt.float32
    AF = mybir.ActivationFunctionType
    OP = mybir.AluOpType
    use_bf16 = MM_DTYPE == "bf16"
    # dtype for tensors consumed by the tensor engine (x, weights, h):
    # bf16 runs the matmul at full rate (fp32r measured at 1/4 rate on HW);
    # gate accumulation stays fp32 in PSUM, s/c stay fp32 on DVE/ACT.
    sb_dt = mybir.dt.bfloat16 if use_bf16 else mybir.dt.float32r

    def R(ap):
        return ap

    nc = bacc.Bacc("TRN2", target_bir_lowering=False, debug=False,
                   num_devices=NCORES)

    xin = nc.dram_tensor("xin", [KX, T * BL], sb_dt, kind="ExternalInput").ap()
    wx = {}
    wu = {}
    wo = {}
    for d in "fb":
        wx[d] = nc.dram_tensor(f"wx_{d}", [KX, G4], sb_dt,
                               kind="ExternalInput").ap()
        wu[d] = nc.dram_tensor(f"wu_{d}", [HID, G4], sb_dt,
                               kind="ExternalInput").ap()
        wo[d] = nc.dram_tensor(f"wo_{d}", [HID, NCLS], sb_dt,
                               kind="ExternalInput").ap()
    out = nc.dram_tensor("out", [NCLS, BL], f32, kind="ExternalOutput").ap()

    with tile.TileContext(nc) as tc, ExitStack() as ctx:
        const = ctx.enter_context(tc.tile_pool(name="const", bufs=1))
        X = const.tile([KX, T * BL], sb_dt, tag="X")
        # split the big input DMA into chunks so it spreads across DMA
        # queues and so early timesteps unblock compute quickly; issue
        # from both ends since the bwd direction consumes t=T-1 first.
        NCHUNK = 16
        CW = T * BL // NCHUNK
        order = []
        for i in range(NCHUNK // 2):
            order += [NCHUNK - 1 - i, i]
        for ci in order:
            nc.sync.dma_start(X[:, ci * CW:(ci + 1) * CW],
                              xin[:, ci * CW:(ci + 1) * CW])

        WX = {}
        WU = {}
        WO = {}
        for d in "fb":
            WX[d] = const.tile([KX, G4], sb_dt, tag=f"wx{d}", name=f"WX{d}")
            nc.sync.dma_start(WX[d][:], wx[d][:])
            WU[d] = const.tile([HID, G4], sb_dt, tag=f"wu{d}", name=f"WU{d}")
            nc.sync.dma_start(WU[d][:], wu[d][:])
            WO[d] = const.tile([HID, NCLS], sb_dt, tag=f"wo{d}", name=f"WO{d}")
            nc.sync.dma_start(WO[d][:], wo[d][:])

        hpool = ctx.enter_context(tc.tile_pool(name="h", bufs=3))
        cpool = ctx.enter_context(tc.tile_pool(name="c", bufs=3))
        spool = ctx.enter_context(tc.tile_pool(name="s", bufs=2))
        scpool = ctx.enter_context(tc.tile_pool(name="sc", bufs=2))
        mpool = ctx.enter_context(tc.tile_pool(name="m1h", bufs=2))
        tpool = ctx.enter_context(tc.tile_pool(name="tt", bufs=2))
        gpsum = ctx.enter_context(tc.tile_pool(name="gates", bufs=1,
                                               space="PSUM"))
        ppsum = ctx.enter_context(tc.tile_pool(name="pacc", bufs=1,
                                               space="PSUM"))
        fpsum = ctx.enter_context(tc.tile_pool(name="fill", bufs=1,
                                               space="PSUM"))

        h = {}
        c = {}
        pacc = {}
        for d in "fb":
            h[d] = hpool.tile([HID, BL], sb_dt, tag=f"h{d}", name=f"h0{d}")
            nc.vector.memset(h[d][:].bitcast(f32) if not use_bf16 else h[d][:],
                             0.0)
            c[d] = cpool.tile([HID, BL], f32, tag=f"c{d}", name=f"c0{d}")
            nc.vector.memset(c[d][:], 0.0)
            pacc[d] = ppsum.tile([NCLS, BL], f32, tag=f"p{d}", name=f"pacc{d}")

        for t in range(T):
            for d in ("f", "b"):
                xi = t if d == "f" else T - 1 - t
                xs = X[:, xi * BL:(xi + 1) * BL]
                g = gpsum.tile([HID, 4 * BL], f32, tag=f"g{d}", name=f"g_{d}_{t}")
                for gi in range(4):
                    nc.tensor.matmul(g[:, gi * BL:(gi + 1) * BL],
                                     R(WX[d][:, gi * HID:(gi + 1) * HID]),
                                     R(xs), start=True, stop=False)
                    nc.tensor.matmul(g[:, gi * BL:(gi + 1) * BL],
                                     R(WU[d][:, gi * HID:(gi + 1) * HID]),
                                     R(h[d][:]), start=False, stop=True)
                # s = sigmoid over all four gates (g-gate pre-doubled)
                s = spool.tile([HID, 4 * BL], f32, tag=f"s{d}")
                nc.scalar.activation(s[:], g[:], AF.Sigmoid)
                s_i = s[:, 0:BL]
                s_f = s[:, BL:2 * BL]
                s_o = s[:, 2 * BL:3 * BL]
                s_g = s[:, 3 * BL:4 * BL]
                # tt = sigmoid(f) * chalf_prev  (c stored as c/2)
                tt = tpool.tile([HID, BL], f32, tag=f"tt{d}")
                nc.vector.tensor_tensor(tt[:], s_f, c[d][:], op=OP.mult)
                # m1h = (s_g - 0.5) * s_i  == 0.5*sigmoid(i)*tanh(g)
                m1h = mpool.tile([HID, BL], f32, tag=f"m{d}")
                nc.vector.scalar_tensor_tensor(m1h[:], s_g, 0.5, s_i,
                                               op0=OP.subtract, op1=OP.mult)
                # chalf = m1h + tt   (== c/2: m1h is 0.5*i*tanh(g),
                # tt is sig(f)*c_prev/2)
                c_new = cpool.tile([HID, BL], f32, tag=f"c{d}")
                nc.vector.tensor_tensor(c_new[:], m1h[:], tt[:], op=OP.add)
                # sc = sigmoid(4*chalf) = sigmoid(2c)
                sc = scpool.tile([HID, BL], f32, tag=f"sc{d}")
                nc.scalar.activation(sc[:], c_new[:], AF.Sigmoid, scale=4.0)
                h_new = hpool.tile([HID, BL], sb_dt, tag=f"h{d}")
                nc.vector.scalar_tensor_tensor(h_new[:], sc[:], 0.5, s_o,
                                               op0=OP.subtract, op1=OP.mult)
                # pool: pacc += (2/T*Wo) @ h_half
                nc.tensor.matmul(pacc[d][:], R(WO[d][:]), R(h_new[:]),
                                 start=(t == 0), stop=(t == T - 1),
                                 skip_group_check=True)
                if USE_FILLER and t < T - 1:
                    # keep the PE HAM activity monitor from re-throttling
                    # during the sigmoid/DVE phase: a throwaway matmul paced
                    # by this step's h (result unused; bank overwritten)
                    fl = fpsum.tile([HID, BL], f32, tag="fill", name=f"fl{d}{t}")
                    nc.tensor.matmul(fl[:], R(WU[d][:, 0:HID]), R(h_new[:]),
                                     start=True, stop=True,
                                     skip_group_check=True)
                h[d] = h_new
                c[d] = c_new

        osum = const.tile([NCLS, BL], f32, tag="osum")
        nc.scalar.copy(osum[:], pacc["f"][:])
        nc.vector.tensor_tensor(osum[:], osum[:], pacc["b"][:], op=OP.add)
        nc.sync.dma_start(out[:], osum[:])

    nc.compile()
    return nc


def _prep_weights(w_ih, w_hh, b_ih, b_hh, fc_in_w, fc_in_b):
    Wx = w_ih.astype(np.float64) @ fc_in_w.astype(np.float64)   # [512, 68]
    bias = w_ih.astype(np.float64) @ fc_in_b.astype(np.float64) \
        + b_ih.astype(np.float64) + b_hh.astype(np.float64)
    perm = np.concatenate([np.arange(0, 128), np.arange(128, 256),
                           np.arange(384, 512), np.arange(256, 384)])
    Wx = Wx[perm]
    U = w_hh.astype(np.float64)[perm]
    bias = bias[perm]
    srow = np.ones((512, 1), np.float64)
    srow[384:] = 2.0
    Wx_aug = np.concatenate([Wx, bias[:, None]], axis=1)        # [512, 69]
    lhsT_x = np.ascontiguousarray((srow * Wx_aug).T)            # [69, 512]
    lhsT_U = np.ascontiguousarray((srow * U * 2.0).T)           # [128, 512]
    return lhsT_x, lhsT_U


def kernel(x, fc_in_w, fc_in_b, w_ih_f, w_hh_f, b_ih_f, b_hh_f,
           w_ih_b, w_hh_b, b_ih_b, b_hh_b, fc_out_w, fc_out_b,
           _want_trace=False):
    from concourse import bass_utils

    np_dt = np.float32
    if MM_DTYPE == "bf16":
        import ml_dtypes
        np_dt = ml_dtypes.bfloat16

    if "nc" not in _CACHE:
        _CACHE["nc"] = _build_program()
    nc = _CACHE["nc"]

    lx_f, lU_f = _prep_weights(w_ih_f, w_hh_f, b_ih_f, b_hh_f,
                               fc_in_w, fc_in_b)
    lx_b, lU_b = _prep_weights(w_ih_b, w_hh_b, b_ih_b, b_hh_b,
                               fc_in_w, fc_in_b)
    wo_f = np.ascontiguousarray((2.0 / T) * fc_out_w[:, :HID].astype(np.float64).T)
    wo_b = np.ascontiguousarray((2.0 / T) * fc_out_w[:, HID:].astype(np.float64).T)

    shared = {
        "wx_f": lx_f.astype(np_dt), "wu_f": lU_f.astype(np_dt),
        "wx_b": lx_b.astype(np_dt), "wu_b": lU_b.astype(np_dt),
        "wo_f": wo_f.astype(np_dt), "wo_b": wo_b.astype(np_dt),
    }
    in_maps = []
    for cidx in range(NCORES):
        xs = x[cidx * BL:(cidx + 1) * BL]                    # [BL, T, 68]
        xT = np.ascontiguousarray(xs.transpose(2, 1, 0))     # [68, T, BL]
        x_aug = np.concatenate(
            [xT, np.ones((1, T, BL), np.float32)], axis=0)   # [69, T, BL]
        in_maps.append({"xin": x_aug.reshape(KX, T * BL).astype(np_dt),
                        **shared})

    res = bass_utils.run_bass_kernel_spmd(
        nc, in_maps, core_ids=list(range(NCORES)), trace=_want_trace)
    outs = []
    for cidx in range(NCORES):
        o = res.results[cidx]["out"]                          # [3, BL]
        out_core = o.T + fc_out_b                             # [BL, 3]
        outs.append(out_core)
    full = np.concatenate(outs, axis=0).astype(np.float32)
    if _want_trace:
        _CACHE["last_result"] = res
    return full
